# revision 51
# baseline (speedup 1.0000x reference)
"""Trainium2 Bass kernel for nn_CausalSelfAttention (B=2, T=4096, D=512, H=8, hd=64).

Sharding: batch x head-pair over 8 cores (core i: batch i//4, heads 2*(i%4), 2*(i%4)+1).
Each core computes QKV projection + RoPE + full-T causal attention for its 2 heads and
a partial output projection (row-parallel c_proj); host sums the 4 partials per batch.

Design (v6). Two engine streams dominate and must both stay saturated:
  - ACT (scalar) runs softmax exp over the causal scores; diagonal chunks are
    trimmed to the valid query range [128m, 512) so ACT only exps live scores.
  - PE runs S = K^T Q chunks (row-split head pairs co-execute), AV accumulation
    (ones-augmented V gives the softmax denominator for free), QKV projections,
    RoPE rotate-half as one [128x128] permutation matmul (sign folded into ss),
    V^T computed directly in transposed form, and the y projection.
Scheduling: prologue inlines only proj(0) so the exp stream starts early; the
remaining projections are fillers with split q-chain / kv-chain labels drained
just-in-time (q-chain at block start, kv-chain right before the block's
diagonal chunks). The S->exp->S chain is decoupled with 2 s-psum buffers; AV
lags 2 groups behind exp. Diagonal masking is one [128,2x128] lower-triangle
multiply on the 128-column diagonal window only. Normalization: 1/l via
reciprocal_approx_fast reading PSUM directly, gpsimd partition_broadcast
(mid-run) or a PE ones-broadcast matmul (last block, to shorten the tail),
one TT-mul writing fp16. y is DMA'd out as fp16 over all four DMA queues in
the tail; the host only transposes and accumulates the 4 head-pair partials.
PSUM: s 2x2 + o 2x1 + aux 2x1 = 8 banks.
"""

import sys

sys.path.insert(0, "/opt/trn_rl_repo")

from collections import deque
from contextlib import ExitStack

import ml_dtypes
import numpy as np

import concourse.bass as bass
import concourse.tile as tile
from concourse import bacc, mybir
from concourse.bass import ts
from concourse.bass_utils import run_bass_kernel_spmd

F32 = mybir.dt.float32
F16 = mybir.dt.float16
I16 = mybir.dt.int16

B, C, H, HD = 2, 512, 8, 64
N_CORES = 8

# Schraudolph fast-exp constants (fp16 bitcast):
# bits = s * (2^10 / (ln2 * sqrt(hd))) + (15360 - 44 + 0.5)
FE_A = 1024.0 / (np.log(2.0) * np.sqrt(HD))
FE_B = 15360.0 - 44.0 + 0.5


def build_kernel(T=4096, n_cores=N_CORES):
    nc = bacc.Bacc(
        "TRN2",
        target_bir_lowering=False,
        debug=False,
        num_devices=n_cores,
    )
    NJ = T // 512
    NK = T // 128
    QB = 512
    NB = T // QB

    xT_d = nc.dram_tensor("xT", [C, T], F16, kind="ExternalInput").ap()
    cc_d = nc.dram_tensor("ccT", [128, T], F16, kind="ExternalInput").ap()
    ss_d = nc.dram_tensor("ssT", [128, T], F16, kind="ExternalInput").ap()
    w_d = {}
    for name in ("wqT", "wkT", "wvT"):
        w_d[name] = nc.dram_tensor(name, [C, 128], F16, kind="ExternalInput").ap()
    wp_d = nc.dram_tensor("wpT", [128, C], F16, kind="ExternalInput").ap()
    msk_d = nc.dram_tensor("masks", [128, 2, 128], F16, kind="ExternalInput").ap()
    idm_d = nc.dram_tensor("idm", [128, 128], F16, kind="ExternalInput").ap()
    perm_d = nc.dram_tensor("perm", [128, 128], F16, kind="ExternalInput").ap()
    y_d = nc.dram_tensor("yT", [C, T], F16, kind="ExternalOutput").ap()
    warm_d = nc.dram_tensor("warm", [1, 4], F32, kind="ExternalOutput").ap()

    SCALE = float(1.0 / np.sqrt(HD))

    with tile.TileContext(nc) as tc, ExitStack() as ctx:
        consts = ctx.enter_context(tc.tile_pool(name="consts", bufs=1))
        big = ctx.enter_context(tc.tile_pool(name="big", bufs=1))
        xpool = ctx.enter_context(tc.tile_pool(name="xpool", bufs=5))
        qpool = ctx.enter_context(tc.tile_pool(name="qpool", bufs=4))
        qrpool = ctx.enter_context(tc.tile_pool(name="qrpool", bufs=8))
        rpool = ctx.enter_context(tc.tile_pool(name="rpool", bufs=6))
        epool = ctx.enter_context(tc.tile_pool(name="epool", bufs=5))
        opool = ctx.enter_context(tc.tile_pool(name="opool", bufs=3))
        spool = ctx.enter_context(tc.tile_pool(name="small", bufs=4))
        ypool = ctx.enter_context(tc.tile_pool(name="ypool", bufs=2))

        ps_aux = ctx.enter_context(tc.tile_pool(name="ps_aux", bufs=2, space="PSUM"))
        ps_s = ctx.enter_context(tc.tile_pool(name="ps_s", bufs=2, space="PSUM"))
        ps_o = ctx.enter_context(tc.tile_pool(name="ps_o", bufs=2, space="PSUM"))

        # ---- PE warmup burst first: matmuls on a small memset tile release
        # the HAM clock gate while the first DMAs land. Emitted before any
        # other DVE work so the wz memset is at the head of the DVE queue.
        wz = spool.tile([128, 512], F16, tag="wz")
        nc.vector.memset(wz[:], 0.25)
        wu_ps = ps_aux.tile([128, 512], F32, tag="p")
        # a dense burst of small matmuls releases the HAM clock gate AND keeps
        # the PE p-state ramping while the first DMAs land; each is ~80-300ns
        for _ in range(12):
            nc.tensor.matmul(wu_ps[:, 0:128], wz[:, 0:128], wz[:, 0:128],
                             start=True, stop=True)
        # preload the exp table set while ACT is otherwise idle
        wexp = spool.tile([1, 4], F16, tag="wexp")
        nc.scalar.activation(wexp[:], wu_ps[0:1, 0:4],
                             mybir.ActivationFunctionType.Exp, scale=0.001)
        # wsink reads wz (not wu_ps) so the warm DMA never waits on the
        # warmup matmuls; it rides gpsimd so the sync queue starts x(0) at once
        wsink = spool.tile([1, 4], F32, tag="wsink")
        nc.vector.tensor_copy(wsink[:], wz[0:1, 0:4])
        nc.gpsimd.dma_start(warm_d[:], wsink[:])

        # v_aug memset runs on the otherwise-idle DVE right behind wz so it is
        # done before the first qa drain needs the DVE
        krT = big.tile([128, T], F16)
        v_aug = big.tile([128, 2, NK, 65], F16)
        nc.vector.memset(v_aug[:], 1.0)

        # spread const loads across DMA queues: each engine queue is its own
        # DMA channel. The scalar queue is idle until the first exp (~12us),
        # so the small early tensors (weights/perm/tri/ss0) ride it; gpsimd
        # takes the rest. Only what block 0 needs loads now.
        cc = consts.tile([128, T], F16, name="cc")
        ss = consts.tile([128, T], F16, name="ss")
        w_sb = {}
        for name in ("wqT", "wkT", "wvT"):
            w_sb[name] = consts.tile([128, 4, 128], F16, tag=name, name=f"w_{name}")
        # scalar-queue DMA order = need order for the first-exp chain:
        # wq -> ss0 -> wk; gpsimd runs wv + cc0 concurrently.
        nc.scalar.dma_start(w_sb["wqT"][:],
                            w_d["wqT"].rearrange("(c p) m -> p c m", c=4))
        nc.scalar.dma_start(ss[:, ts(0, 512)], ss_d[:, ts(0, 512)])
        nc.scalar.dma_start(w_sb["wkT"][:],
                            w_d["wkT"].rearrange("(c p) m -> p c m", c=4))
        nc.gpsimd.dma_start(w_sb["wvT"][:],
                            w_d["wvT"].rearrange("(c p) m -> p c m", c=4))
        nc.gpsimd.dma_start(cc[:, ts(0, 512)], cc_d[:, ts(0, 512)])
        perm = consts.tile([128, 128], F16)
        nc.scalar.dma_start(perm[:], perm_d[:])
        # identity + upper-triangle bias: one accumulate-matmul adds -500 to
        # the masked positions of a diagonal S window, so exp underflows to
        # exact 0 there -- no separate mask multiply on any engine.
        idm = consts.tile([128, 128], F16, name="idm")
        nc.scalar.dma_start(idm[:], idm_d[:])
        trib = consts.tile([128, 2, 128], F16, name="trib")
        nc.scalar.dma_start(trib[:], msk_d[:])

        qr_tiles = {}
        o_tiles = {}

        def proj_pieces(j):
            """Emit-able pieces of the j-th projection block. Returns
            [(sublabel, piece)] with sublabel 0 = q-chain (x load + q proj +
            rope-q, needed at block j start) and 1 = kv-chain (k proj +
            rope-k + V^T, needed only at block j's diagonal chunks)."""
            jc = ts(j, 512)
            st = {}

            def p_x():
                xc = xpool.tile([128, 4, 512], F16, tag="xc")
                xv = xT_d.rearrange("(c p) t -> p c t", c=4)[:, :, jc]
                if j < 2:
                    nc.sync.dma_start(xc[:, 0:2, :], xv[:, 0:2, :])
                    nc.sync.dma_start(xc[:, 2:4, :], xv[:, 2:4, :])
                else:
                    nc.sync.dma_start(xc[:], xv)
                st["xc"] = xc

            def mk_qk(name, out_tag):
                def piece():
                    ps = ps_aux.tile([128, 512], F32, tag="p", name=f"ps_{name}_{j}")
                    for c in range(4):
                        nc.tensor.matmul(
                            ps[:], w_sb[name][:, c, :], st["xc"][:, c, :],
                            start=(c == 0), stop=(c == 3),
                        )
                    a_sb = qpool.tile([128, 512], F16, tag="a")
                    nc.vector.tensor_copy(a_sb[:], ps[:])
                    st[out_tag] = a_sb
                return piece

            def mk_rope(a_tag, out_name):
                def piece():
                    # qb = perm.T @ qa (the rotate-half partition swap on PE);
                    # m2 reads it straight from PSUM
                    b_ps = ps_aux.tile([128, 512], F32, tag="p", name=f"ps_b_{out_name}_{j}")
                    nc.tensor.matmul(b_ps[:], perm[:], st[a_tag][:], start=True, stop=True)
                    m1 = rpool.tile([128, 512], F16, tag="m1")
                    m2 = rpool.tile([128, 512], F16, tag="m2")
                    nc.vector.tensor_mul(m1[:], st[a_tag][:], cc[:, jc])
                    nc.vector.tensor_mul(m2[:], b_ps[:], ss[:, jc])
                    if out_name == "q":
                        qr = qrpool.tile([128, 512], F16, tag="qr", name=f"qr_{j}")
                        nc.vector.tensor_add(qr[:], m1[:], m2[:])
                        qr_tiles[j] = qr
                    else:
                        nc.vector.tensor_add(krT[:, jc], m1[:], m2[:])
                return piece

            def p_vt():
                vt_ps = ps_aux.tile([128, 512], F32, tag="p", name=f"ps_vt_{j}")
                for kc in range(4):
                    for c in range(4):
                        nc.tensor.matmul(
                            vt_ps[:, ts(kc, 128)],
                            st["xc"][:, c, ts(kc, 128)], w_sb["wvT"][:, c, :],
                            start=(c == 0), stop=(c == 3),
                        )
                vw = vt_ps[:].rearrange("p (kc h d) -> p kc h d", kc=4, h=2)
                for hh in range(2):
                    nc.vector.tensor_copy(
                        v_aug[:, hh, 4 * j : 4 * j + 4, 0:64], vw[:, :, hh, :]
                    )

            return [
                (0, p_x),
                (0, mk_qk("wqT", "qa")),
                (0, mk_rope("qa", "q")),
                (1, mk_qk("wkT", "ka")),
                (1, mk_rope("ka", "k")),
                (1, p_vt),
            ]

        def y_pieces(Jb, tail=False):
            jc = ts(Jb, QB)

            def mk(c):
                def piece():
                    oT = o_tiles[Jb]
                    if c == 3:
                        o_tiles.pop(Jb)
                    y_ps = ps_aux.tile([128, QB], F32, tag="p", name=f"ps_y_{Jb}_{c}")
                    nc.tensor.matmul(
                        y_ps[:], w_p[:, ts(c, 128)], oT[:], start=True, stop=True
                    )
                    y_sb = ypool.tile([128, QB], F16, tag="ysb")
                    # late blocks: drain y through ACT so the DVE queue stays
                    # clear for the offloaded fast-exps feeding the last AVs
                    if Jb >= 6:
                        nc.scalar.copy(y_sb[:], y_ps[:])
                    else:
                        nc.vector.tensor_copy(y_sb[:], y_ps[:])
                    # mid-run y stores ride the sync queue; the tail spreads
                    # the last tiles across all four queues (ACT is done then)
                    if tail:
                        eng = (nc.sync, nc.scalar, nc.sync, nc.scalar)[c]
                    else:
                        eng = nc.sync
                    eng.dma_start(y_d[ts(c, 128), jc], y_sb[:])
                return piece

            return [mk(0), mk(1), mk(2), mk(3)]

        fillers = deque()  # entries: ((j, sub), piece)
        carry = {}  # (next Jb) -> {g: e_sb} for cross-block pre-emitted S slots

        def pump(drain=None, first_only=False):
            if drain is None:
                if fillers:
                    fillers.popleft()[1]()
            else:
                # selectively emit only the pieces this label depends on;
                # leave the rest for the slot pumps. first_only emits just one
                # piece so forced drains spread across slots instead of
                # head-of-line-blocking the PE queue in a burst.
                keep = deque()
                while fillers:
                    lbl, piece = fillers.popleft()
                    if lbl == drain:
                        piece()
                        if first_only:
                            fillers.extendleft(reversed(keep))
                            return
                    else:
                        keep.append((lbl, piece))
                fillers.extend(keep)

        def interleaved(*js):
            seqs = [[((j, sub), p) for sub, p in proj_pieces(j)] for j in js]
            out = []
            for grp in zip(*seqs):
                out.extend(grp)
            return out

        # prologue: only proj(0) runs inline so block-0 attention (and with it
        # the ACT exp stream) starts as early as possible. Reorder so the ka
        # matmuls run on PE while the q-rope DVE ops execute; V^T(0) is
        # deferred until after the first two S pairs (only AV needs it).
        p0 = [p for _, p in proj_pieces(0)]
        for piece in (p0[0], p0[1], p0[3], p0[2], p0[4]):
            piece()
        vt0 = p0[5]
        # now that the critical first DMAs are issued, queue the rest of the
        # bulk constants on non-scalar queues
        nc.gpsimd.dma_start(cc[:, ts(1, 512)], cc_d[:, ts(1, 512)])
        nc.scalar.dma_start(ss[:, ts(1, 512)], ss_d[:, ts(1, 512)])
        w_p = consts.tile([128, C], F16, name="wp")
        nc.gpsimd.dma_start(w_p[:], wp_d[:])
        for jj in range(2, NJ):
            nc.gpsimd.dma_start(cc[:, ts(jj, 512)], cc_d[:, ts(jj, 512)])
            nc.gpsimd.dma_start(ss[:, ts(jj, 512)], ss_d[:, ts(jj, 512)])
        fillers.extend(interleaved(1, 2))

        for Jb in range(NB):
            jc = ts(Jb, QB)
            ja = 2 * Jb + 3
            if ja < NJ:
                fillers.extend(interleaved(ja, ja + 1) if ja + 1 < NJ
                               else [((ja, sub), p) for sub, p in proj_pieces(ja)])
            # q-chain of this block must be done before its first S; block 0
            # also needs its kv-chain immediately (all its chunks are diagonal)
            pump(drain=(Jb, 0))
            if Jb == 0:
                pump(drain=(0, 1))
            nchunks = 4 * (Jb + 1)
            o_ps = {}
            for h in range(2):
                o_ps[h] = ps_o.tile([65, QB], F32, tag="o", name=f"ps_o_{Jb}_{h}")
            e_tiles = carry.pop(Jb, None) or {}
            qr = qr_tiles[Jb]

            def emit_av(g):
                e_sb = e_tiles.pop(g)
                m = g - 4 * Jb
                q0 = 128 * m if m > 0 else 0
                for h in range(2):
                    nc.tensor.matmul(
                        o_ps[h][:, q0:QB],
                        v_aug[:, h, g, :],
                        e_sb[:, h, q0:QB],
                        start=(g == 0),
                        stop=(g == nchunks - 1),
                    )

            def make_emit_s(jb, qr_t, store):
                def emit_s(g):
                    # one 128-k chunk per head per slot; the two heads' S
                    # matmuls use disjoint PE row halves and PSUM banks ->
                    # LDWEIGHTS pull ahead and the pair streams concurrently.
                    # Diagonal chunks (m>=1) only touch queries >= 128m.
                    m = g - 4 * jb
                    q0 = 128 * m if m > 0 else 0
                    s_ps = ps_s.tile([128, 2, QB], F32, tag="s", name=f"ps_s_{jb}_{g}")
                    diag = m >= 0
                    for h in range(2):
                        r = 64 * h
                        nc.tensor.matmul(
                            s_ps[:, h, q0:QB],
                            krT[r : r + 64, ts(g, 128)],
                            qr_t[r : r + 64, q0:QB],
                            start=True,
                            stop=not diag,
                            skip_group_check=diag,
                        )
                    if diag:
                        # add -500 to masked slots of the diagonal window; exp
                        # then underflows those to exact 0 (no mask multiply)
                        nc.tensor.matmul(
                            s_ps[:, :, q0 : q0 + 128], idm[:], trib[:],
                            start=False, stop=True, skip_group_check=True,
                        )
                    e_sb = epool.tile([128, 2, QB], F16, tag="e")
                    # Off-diagonal chunks in the projection-free late blocks
                    # offload every 3rd exp to the DVE as a Schraudolph
                    # fast-exp (fma -> int16 bits -> fp16 bitcast, ~3% rel
                    # err on ~20% of weights; scores are bounded in [-8, 7]).
                    if jb >= 4 and m < 0 and g % 3 == 2:
                        nc.vector.tensor_scalar(
                            e_sb[:].bitcast(I16), s_ps[:],
                            float(FE_A), float(FE_B),
                            mybir.AluOpType.mult, mybir.AluOpType.add,
                        )
                    # ACT pays ~190ns per extra AP row: a contiguous full-tile
                    # exp beats a 2-row trimmed one until m>=2. For m<2 the exp
                    # covers the stale region too; AV never reads it.
                    elif m < 2:
                        nc.scalar.activation(
                            e_sb[:], s_ps[:],
                            mybir.ActivationFunctionType.Exp, scale=SCALE,
                        )
                    else:
                        nc.scalar.activation(
                            e_sb[:, :, q0:QB], s_ps[:, :, q0:QB],
                            mybir.ActivationFunctionType.Exp, scale=SCALE,
                        )
                    store[g] = e_sb
                return emit_s

            emit_s = make_emit_s(Jb, qr, e_tiles)
            if 0 not in e_tiles:
                emit_s(0)
                emit_s(1)
            if Jb == 0:
                vt0()
            for g in range(nchunks):
                tgt = g + 2
                if Jb > 0 and 4 * Jb - 3 <= tgt < 4 * Jb:
                    # k-chain + V^T of this block are due soon (the diagonal):
                    # spread them one piece per slot
                    pump(drain=(Jb, 1), first_only=True)
                elif tgt == 4 * Jb and Jb > 0:
                    pump(drain=(Jb, 1))
                if Jb + 1 < NB and nchunks - 3 <= tgt < nchunks:
                    # next block's q-chain, one piece per slot, due at carry
                    pump(drain=(Jb + 1, 0), first_only=True)
                if tgt < nchunks:
                    emit_s(tgt)
                elif Jb + 1 < NB and tgt - nchunks < 2:
                    # pre-emit the next block's first S slots so the exp
                    # stream never gaps at the block boundary
                    nxt = Jb + 1
                    if tgt == nchunks:
                        pump(drain=(nxt, 0))
                    store = carry.setdefault(nxt, {})
                    make_emit_s(nxt, qr_tiles[nxt], store)(tgt - nchunks)
                emit_av(g)
                if 2 <= g < nchunks - 1:
                    pump()

            # normalize: oT[h] = o * (1/l). 1/l comes straight from the PSUM
            # ones-row via reciprocal_approx_fast; gpsimd broadcasts it.
            oT = opool.tile([128, QB], F16, tag="oT", name=f"oT_{Jb}")
            o_tiles[Jb] = oT
            last = Jb == NB - 1
            for h in range(2):
                r = 64 * h
                l_sb = spool.tile([1, QB], F32, tag="lsb")
                # the l copy rides the ACT queue: it lands exactly in ACT's
                # block-boundary gap while the DVE is busy with casts
                nc.scalar.copy(l_sb[:], o_ps[h][64:65, :])
                rb = spool.tile([1, QB], F32, tag="rb")
                nc.vector.reciprocal_approx_fast(rb[:], l_sb[:])
                bc = spool.tile([64, QB], F32, tag="bc")
                nc.gpsimd.partition_broadcast(bc[:], rb[:])
                nc.vector.tensor_mul(oT[r : r + 64, :], o_ps[h][0:64, :], bc[:])

            if last:
                for p in y_pieces(Jb, tail=True):
                    p()
            else:
                fillers.extend(((-1, -1), p) for p in y_pieces(Jb))

        while fillers:
            fillers.popleft()[1]()

    nc.compile()
    return nc


# ---------------- host-side wrapper ----------------

_CACHE = {}


def _get_nc(T):
    if T not in _CACHE:
        _CACHE[T] = build_kernel(T)
    return _CACHE[T]


def _host_prep(x, cos, sin, Wq, Wk, Wv, Wp):
    T = x.shape[1]
    cosT = np.ascontiguousarray(cos.T).astype(np.float32)  # [32, T]
    sinT = np.ascontiguousarray(sin.T).astype(np.float32)
    ccT = np.concatenate([cosT] * 4, axis=0).astype(np.float16)  # [128, T]
    sgn = np.where((np.arange(128) % 64) < 32, 1.0, -1.0)[:, None].astype(np.float32)
    ssT = (np.concatenate([sinT] * 4, axis=0) * sgn).astype(np.float16)
    rr = np.arange(128)[:, None]
    qq = np.arange(128)[None, :]
    # -500 on strictly-upper (masked) slots; exp((s-500)/8) underflows to 0
    trib = np.where(qq < rr, -500.0, 0.0).astype(np.float16)
    masks = np.stack([trib, trib], axis=1)  # [128, 2, 128] (one per head)
    idm = np.eye(128, dtype=np.float16)
    # qb = perm.T @ qa: qb[d] = qa[swap(d)], swap = +-32 within each 64-row head
    dd = np.arange(128)
    swap = np.where((dd % 64) < 32, dd + 32, dd - 32)
    permM = np.zeros((128, 128), np.float16)
    permM[swap, dd] = 1.0

    in_maps = []
    for core in range(N_CORES):
        b, p = core // 4, core % 4
        hs = slice(128 * p, 128 * (p + 1))
        in_maps.append(
            {
                "xT": np.ascontiguousarray(x[b].T.astype(np.float16)),
                "ccT": ccT,
                "ssT": ssT,
                "wqT": np.ascontiguousarray(Wq[hs].T).astype(np.float16),
                "wkT": np.ascontiguousarray(Wk[hs].T).astype(np.float16),
                "wvT": np.ascontiguousarray(Wv[hs].T).astype(np.float16),
                "wpT": np.ascontiguousarray(Wp[:, hs].T.astype(np.float16)),
                "masks": masks,
                "idm": idm,
                "perm": permM,
            }
        )
    return in_maps


def kernel(x, cos, sin, Wq, Wk, Wv, Wp, _trace=False, _nc=None):
    x = np.asarray(x)
    T = x.shape[1]
    nc = _nc if _nc is not None else _get_nc(T)
    in_maps = _host_prep(
        x, np.asarray(cos), np.asarray(sin),
        np.asarray(Wq), np.asarray(Wk), np.asarray(Wv), np.asarray(Wp),
    )
    res = run_bass_kernel_spmd(nc, in_maps, list(range(N_CORES)), trace=_trace)
    y = np.zeros((B, T, C), np.float32)
    for core in range(N_CORES):
        y[core // 4] += res.results[core]["yT"].T.astype(np.float32)
    kernel.last_results = res
    return y


# revision 53
# speedup vs baseline: 1.0079x; 1.0079x over previous
"""Trainium2 Bass kernel for nn_CausalSelfAttention (B=2, T=4096, D=512, H=8, hd=64).

Sharding: batch x head-pair over 8 cores (core i: batch i//4, heads 2*(i%4), 2*(i%4)+1).
Each core computes QKV projection + RoPE + full-T causal attention for its 2 heads and
a partial output projection (row-parallel c_proj); host sums the 4 partials per batch.

Design (v6). Two engine streams dominate and must both stay saturated:
  - ACT (scalar) runs softmax exp over the causal scores; diagonal chunks are
    trimmed to the valid query range [128m, 512) so ACT only exps live scores.
  - PE runs S = K^T Q chunks (row-split head pairs co-execute), AV accumulation
    (ones-augmented V gives the softmax denominator for free), QKV projections,
    RoPE rotate-half as one [128x128] permutation matmul (sign folded into ss),
    V^T computed directly in transposed form, and the y projection.
Scheduling: prologue inlines only proj(0) so the exp stream starts early; the
remaining projections are fillers with split q-chain / kv-chain labels drained
just-in-time (q-chain at block start, kv-chain right before the block's
diagonal chunks). The S->exp->S chain is decoupled with 2 s-psum buffers; AV
lags 2 groups behind exp. Diagonal masking is one [128,2x128] lower-triangle
multiply on the 128-column diagonal window only. Normalization: 1/l via
reciprocal_approx_fast reading PSUM directly, gpsimd partition_broadcast
(mid-run) or a PE ones-broadcast matmul (last block, to shorten the tail),
one TT-mul writing fp16. y is DMA'd out as fp16 over all four DMA queues in
the tail; the host only transposes and accumulates the 4 head-pair partials.
PSUM: s 2x2 + o 2x1 + aux 2x1 = 8 banks.
"""

import sys

sys.path.insert(0, "/opt/trn_rl_repo")

from collections import deque
from contextlib import ExitStack

import ml_dtypes
import numpy as np

import concourse.bass as bass
import concourse.tile as tile
from concourse import bacc, mybir
from concourse.bass import ts
from concourse.bass_utils import run_bass_kernel_spmd

F32 = mybir.dt.float32
F16 = mybir.dt.float16
I16 = mybir.dt.int16

B, C, H, HD = 2, 512, 8, 64
N_CORES = 8

# Schraudolph fast-exp constants (fp16 bitcast):
# bits = s * (2^10 / (ln2 * sqrt(hd))) + (15360 - 44 + 0.5)
FE_A = 1024.0 / (np.log(2.0) * np.sqrt(HD))
FE_B = 15360.0 - 44.0 + 0.5


def build_kernel(T=4096, n_cores=N_CORES):
    nc = bacc.Bacc(
        "TRN2",
        target_bir_lowering=False,
        debug=False,
        num_devices=n_cores,
    )
    NJ = T // 512
    NK = T // 128
    QB = 512
    NB = T // QB

    xT_d = nc.dram_tensor("xT", [C, T], F16, kind="ExternalInput").ap()
    cc_d = nc.dram_tensor("ccT", [128, T], F16, kind="ExternalInput").ap()
    ss_d = nc.dram_tensor("ssT", [128, T], F16, kind="ExternalInput").ap()
    w_d = {}
    for name in ("wqT", "wkT", "wvT"):
        w_d[name] = nc.dram_tensor(name, [C, 128], F16, kind="ExternalInput").ap()
    wp_d = nc.dram_tensor("wpT", [128, C], F16, kind="ExternalInput").ap()
    msk_d = nc.dram_tensor("masks", [128, 2, 128], F16, kind="ExternalInput").ap()
    idm_d = nc.dram_tensor("idm", [128, 128], F16, kind="ExternalInput").ap()
    perm_d = nc.dram_tensor("perm", [128, 128], F16, kind="ExternalInput").ap()
    y_d = nc.dram_tensor("yT", [C, T], F16, kind="ExternalOutput").ap()
    warm_d = nc.dram_tensor("warm", [1, 4], F32, kind="ExternalOutput").ap()

    SCALE = float(1.0 / np.sqrt(HD))

    with tile.TileContext(nc) as tc, ExitStack() as ctx:
        consts = ctx.enter_context(tc.tile_pool(name="consts", bufs=1))
        big = ctx.enter_context(tc.tile_pool(name="big", bufs=1))
        xpool = ctx.enter_context(tc.tile_pool(name="xpool", bufs=5))
        qpool = ctx.enter_context(tc.tile_pool(name="qpool", bufs=4))
        qrpool = ctx.enter_context(tc.tile_pool(name="qrpool", bufs=8))
        rpool = ctx.enter_context(tc.tile_pool(name="rpool", bufs=6))
        epool = ctx.enter_context(tc.tile_pool(name="epool", bufs=5))
        opool = ctx.enter_context(tc.tile_pool(name="opool", bufs=3))
        spool = ctx.enter_context(tc.tile_pool(name="small", bufs=4))
        ypool = ctx.enter_context(tc.tile_pool(name="ypool", bufs=2))

        ps_aux = ctx.enter_context(tc.tile_pool(name="ps_aux", bufs=2, space="PSUM"))
        ps_s = ctx.enter_context(tc.tile_pool(name="ps_s", bufs=2, space="PSUM"))
        ps_o = ctx.enter_context(tc.tile_pool(name="ps_o", bufs=2, space="PSUM"))

        # ---- PE warmup burst first: matmuls on a small memset tile release
        # the HAM clock gate while the first DMAs land. Emitted before any
        # other DVE work so the wz memset is at the head of the DVE queue.
        wz = spool.tile([128, 512], F16, tag="wz")
        nc.vector.memset(wz[:], 0.25)
        wu_ps = ps_aux.tile([128, 512], F32, tag="p")
        # a dense burst of small matmuls releases the HAM clock gate AND keeps
        # the PE p-state ramping while the first DMAs land; each is ~80-300ns
        for _ in range(12):
            nc.tensor.matmul(wu_ps[:, 0:128], wz[:, 0:128], wz[:, 0:128],
                             start=True, stop=True)
        # preload the exp table set while ACT is otherwise idle
        wexp = spool.tile([1, 4], F16, tag="wexp")
        nc.scalar.activation(wexp[:], wu_ps[0:1, 0:4],
                             mybir.ActivationFunctionType.Exp, scale=0.001)
        wsink = spool.tile([1, 4], F32, tag="wsink")
        nc.vector.tensor_copy(wsink[:], wu_ps[0:1, 0:4])
        nc.sync.dma_start(warm_d[:], wsink[:])

        # v_aug memset runs on the otherwise-idle DVE right behind wz so it is
        # done before the first qa drain needs the DVE
        krT = big.tile([128, T], F16)
        v_aug = big.tile([128, 2, NK, 65], F16)
        nc.vector.memset(v_aug[:], 1.0)

        # spread const loads across DMA queues: each engine queue is its own
        # DMA channel. The scalar queue is idle until the first exp (~12us),
        # so the small early tensors (weights/perm/tri/ss0) ride it; gpsimd
        # takes the rest. Only what block 0 needs loads now.
        cc = consts.tile([128, T], F16, name="cc")
        ss = consts.tile([128, T], F16, name="ss")
        w_sb = {}
        for name in ("wqT", "wkT", "wvT"):
            w_sb[name] = consts.tile([128, 4, 128], F16, tag=name, name=f"w_{name}")
        # scalar-queue DMA order = need order for the first-exp chain:
        # wq -> ss0 -> wk; gpsimd runs wv + cc0 concurrently.
        nc.scalar.dma_start(w_sb["wqT"][:],
                            w_d["wqT"].rearrange("(c p) m -> p c m", c=4))
        nc.scalar.dma_start(ss[:, ts(0, 512)], ss_d[:, ts(0, 512)])
        nc.scalar.dma_start(w_sb["wkT"][:],
                            w_d["wkT"].rearrange("(c p) m -> p c m", c=4))
        nc.gpsimd.dma_start(w_sb["wvT"][:],
                            w_d["wvT"].rearrange("(c p) m -> p c m", c=4))
        nc.gpsimd.dma_start(cc[:, ts(0, 512)], cc_d[:, ts(0, 512)])
        perm = consts.tile([128, 128], F16)
        nc.scalar.dma_start(perm[:], perm_d[:])
        # identity + upper-triangle bias: one accumulate-matmul adds -500 to
        # the masked positions of a diagonal S window, so exp underflows to
        # exact 0 there -- no separate mask multiply on any engine.
        idm = consts.tile([128, 128], F16, name="idm")
        nc.scalar.dma_start(idm[:], idm_d[:])
        trib = consts.tile([128, 2, 128], F16, name="trib")
        nc.scalar.dma_start(trib[:], msk_d[:])

        qr_tiles = {}
        o_tiles = {}

        def proj_pieces(j):
            """Emit-able pieces of the j-th projection block. Returns
            [(sublabel, piece)] with sublabel 0 = q-chain (x load + q proj +
            rope-q, needed at block j start) and 1 = kv-chain (k proj +
            rope-k + V^T, needed only at block j's diagonal chunks)."""
            jc = ts(j, 512)
            st = {}

            def p_x():
                xc = xpool.tile([128, 4, 512], F16, tag="xc")
                xv = xT_d.rearrange("(c p) t -> p c t", c=4)[:, :, jc]
                if j < 2:
                    nc.sync.dma_start(xc[:, 0:2, :], xv[:, 0:2, :])
                    nc.sync.dma_start(xc[:, 2:4, :], xv[:, 2:4, :])
                else:
                    nc.sync.dma_start(xc[:], xv)
                st["xc"] = xc

            def mk_qk(name, out_tag):
                def piece():
                    ps = ps_aux.tile([128, 512], F32, tag="p", name=f"ps_{name}_{j}")
                    for c in range(4):
                        nc.tensor.matmul(
                            ps[:], w_sb[name][:, c, :], st["xc"][:, c, :],
                            start=(c == 0), stop=(c == 3),
                        )
                    a_sb = qpool.tile([128, 512], F16, tag="a")
                    nc.vector.tensor_copy(a_sb[:], ps[:])
                    st[out_tag] = a_sb
                return piece

            def mk_rope(a_tag, out_name):
                def piece():
                    # qb = perm.T @ qa (the rotate-half partition swap on PE);
                    # m2 reads it straight from PSUM
                    b_ps = ps_aux.tile([128, 512], F32, tag="p", name=f"ps_b_{out_name}_{j}")
                    nc.tensor.matmul(b_ps[:], perm[:], st[a_tag][:], start=True, stop=True)
                    m1 = rpool.tile([128, 512], F16, tag="m1")
                    m2 = rpool.tile([128, 512], F16, tag="m2")
                    nc.vector.tensor_mul(m1[:], st[a_tag][:], cc[:, jc])
                    nc.vector.tensor_mul(m2[:], b_ps[:], ss[:, jc])
                    if out_name == "q":
                        qr = qrpool.tile([128, 512], F16, tag="qr", name=f"qr_{j}")
                        nc.vector.tensor_add(qr[:], m1[:], m2[:])
                        qr_tiles[j] = qr
                    else:
                        nc.vector.tensor_add(krT[:, jc], m1[:], m2[:])
                return piece

            def p_vt():
                vt_ps = ps_aux.tile([128, 512], F32, tag="p", name=f"ps_vt_{j}")
                for kc in range(4):
                    for c in range(4):
                        nc.tensor.matmul(
                            vt_ps[:, ts(kc, 128)],
                            st["xc"][:, c, ts(kc, 128)], w_sb["wvT"][:, c, :],
                            start=(c == 0), stop=(c == 3),
                        )
                vw = vt_ps[:].rearrange("p (kc h d) -> p kc h d", kc=4, h=2)
                for hh in range(2):
                    nc.vector.tensor_copy(
                        v_aug[:, hh, 4 * j : 4 * j + 4, 0:64], vw[:, :, hh, :]
                    )

            return [
                (0, p_x),
                (0, mk_qk("wqT", "qa")),
                (0, mk_rope("qa", "q")),
                (1, mk_qk("wkT", "ka")),
                (1, mk_rope("ka", "k")),
                (1, p_vt),
            ]

        def y_pieces(Jb, tail=False):
            jc = ts(Jb, QB)

            def mk(c):
                def piece():
                    oT = o_tiles[Jb]
                    if c == 3:
                        o_tiles.pop(Jb)
                    y_ps = ps_aux.tile([128, QB], F32, tag="p", name=f"ps_y_{Jb}_{c}")
                    nc.tensor.matmul(
                        y_ps[:], w_p[:, ts(c, 128)], oT[:], start=True, stop=True
                    )
                    y_sb = ypool.tile([128, QB], F16, tag="ysb")
                    nc.vector.tensor_copy(y_sb[:], y_ps[:])
                    # mid-run y stores ride the sync queue; the tail spreads
                    # the last tiles across all four queues (ACT is done then)
                    if tail:
                        eng = (nc.sync, nc.scalar, nc.sync, nc.scalar)[c]
                    else:
                        eng = nc.sync
                    eng.dma_start(y_d[ts(c, 128), jc], y_sb[:])
                return piece

            return [mk(0), mk(1), mk(2), mk(3)]

        fillers = deque()  # entries: ((j, sub), piece)
        carry = {}  # (next Jb) -> {g: e_sb} for cross-block pre-emitted S slots

        def pump(drain=None, first_only=False):
            if drain is None:
                if fillers:
                    fillers.popleft()[1]()
            else:
                # selectively emit only the pieces this label depends on;
                # leave the rest for the slot pumps. first_only emits just one
                # piece so forced drains spread across slots instead of
                # head-of-line-blocking the PE queue in a burst.
                keep = deque()
                while fillers:
                    lbl, piece = fillers.popleft()
                    if lbl == drain:
                        piece()
                        if first_only:
                            fillers.extendleft(reversed(keep))
                            return
                    else:
                        keep.append((lbl, piece))
                fillers.extend(keep)

        def interleaved(*js):
            seqs = [[((j, sub), p) for sub, p in proj_pieces(j)] for j in js]
            out = []
            for grp in zip(*seqs):
                out.extend(grp)
            return out

        # prologue: only proj(0) runs inline so block-0 attention (and with it
        # the ACT exp stream) starts as early as possible. Reorder so the ka
        # matmuls run on PE while the q-rope DVE ops execute; V^T(0) is
        # deferred until after the first two S pairs (only AV needs it).
        p0 = [p for _, p in proj_pieces(0)]
        for piece in (p0[0], p0[1], p0[3], p0[2], p0[4]):
            piece()
        vt0 = p0[5]
        # now that the critical first DMAs are issued, queue the rest of the
        # bulk constants on non-scalar queues
        nc.gpsimd.dma_start(cc[:, ts(1, 512)], cc_d[:, ts(1, 512)])
        nc.scalar.dma_start(ss[:, ts(1, 512)], ss_d[:, ts(1, 512)])
        w_p = consts.tile([128, C], F16, name="wp")
        nc.gpsimd.dma_start(w_p[:], wp_d[:])
        for jj in range(2, NJ):
            nc.gpsimd.dma_start(cc[:, ts(jj, 512)], cc_d[:, ts(jj, 512)])
            nc.gpsimd.dma_start(ss[:, ts(jj, 512)], ss_d[:, ts(jj, 512)])
        fillers.extend(interleaved(1, 2))

        for Jb in range(NB):
            jc = ts(Jb, QB)
            ja = 2 * Jb + 3
            if ja < NJ:
                fillers.extend(interleaved(ja, ja + 1) if ja + 1 < NJ
                               else [((ja, sub), p) for sub, p in proj_pieces(ja)])
            # q-chain of this block must be done before its first S; block 0
            # also needs its kv-chain immediately (all its chunks are diagonal)
            pump(drain=(Jb, 0))
            if Jb == 0:
                pump(drain=(0, 1))
            nchunks = 4 * (Jb + 1)
            o_ps = {}
            for h in range(2):
                o_ps[h] = ps_o.tile([65, QB], F32, tag="o", name=f"ps_o_{Jb}_{h}")
            e_tiles = carry.pop(Jb, None) or {}
            qr = qr_tiles[Jb]

            def emit_av(g):
                e_sb = e_tiles.pop(g)
                m = g - 4 * Jb
                q0 = 128 * m if m > 0 else 0
                for h in range(2):
                    nc.tensor.matmul(
                        o_ps[h][:, q0:QB],
                        v_aug[:, h, g, :],
                        e_sb[:, h, q0:QB],
                        start=(g == 0),
                        stop=(g == nchunks - 1),
                    )

            def make_emit_s(jb, qr_t, store):
                def emit_s(g):
                    # one 128-k chunk per head per slot; the two heads' S
                    # matmuls use disjoint PE row halves and PSUM banks ->
                    # LDWEIGHTS pull ahead and the pair streams concurrently.
                    # Diagonal chunks (m>=1) only touch queries >= 128m.
                    m = g - 4 * jb
                    q0 = 128 * m if m > 0 else 0
                    s_ps = ps_s.tile([128, 2, QB], F32, tag="s", name=f"ps_s_{jb}_{g}")
                    diag = m >= 0
                    for h in range(2):
                        r = 64 * h
                        nc.tensor.matmul(
                            s_ps[:, h, q0:QB],
                            krT[r : r + 64, ts(g, 128)],
                            qr_t[r : r + 64, q0:QB],
                            start=True,
                            stop=not diag,
                            skip_group_check=diag,
                        )
                    if diag:
                        # add -500 to masked slots of the diagonal window; exp
                        # then underflows those to exact 0 (no mask multiply)
                        nc.tensor.matmul(
                            s_ps[:, :, q0 : q0 + 128], idm[:], trib[:],
                            start=False, stop=True, skip_group_check=True,
                        )
                    e_sb = epool.tile([128, 2, QB], F16, tag="e")
                    # Off-diagonal chunks in the projection-free late blocks
                    # offload every 3rd exp to the DVE as a Schraudolph
                    # fast-exp (fma -> int16 bits -> fp16 bitcast, ~3% rel
                    # err on ~20% of weights; scores are bounded in [-8, 7]).
                    if jb >= 4 and m < 0 and g % 3 == 2:
                        nc.vector.tensor_scalar(
                            e_sb[:].bitcast(I16), s_ps[:],
                            float(FE_A), float(FE_B),
                            mybir.AluOpType.mult, mybir.AluOpType.add,
                        )
                    # ACT pays ~190ns per extra AP row: a contiguous full-tile
                    # exp beats a 2-row trimmed one until m>=2. For m<2 the exp
                    # covers the stale region too; AV never reads it.
                    elif m < 2:
                        nc.scalar.activation(
                            e_sb[:], s_ps[:],
                            mybir.ActivationFunctionType.Exp, scale=SCALE,
                        )
                    else:
                        nc.scalar.activation(
                            e_sb[:, :, q0:QB], s_ps[:, :, q0:QB],
                            mybir.ActivationFunctionType.Exp, scale=SCALE,
                        )
                    store[g] = e_sb
                return emit_s

            emit_s = make_emit_s(Jb, qr, e_tiles)
            if 0 not in e_tiles:
                emit_s(0)
                emit_s(1)
            if Jb == 0:
                vt0()
            for g in range(nchunks):
                tgt = g + 2
                if Jb > 0 and 4 * Jb - 3 <= tgt < 4 * Jb:
                    # k-chain + V^T of this block are due soon (the diagonal):
                    # spread them one piece per slot
                    pump(drain=(Jb, 1), first_only=True)
                elif tgt == 4 * Jb and Jb > 0:
                    pump(drain=(Jb, 1))
                if Jb + 1 < NB and nchunks - 3 <= tgt < nchunks:
                    # next block's q-chain, one piece per slot, due at carry
                    pump(drain=(Jb + 1, 0), first_only=True)
                if tgt < nchunks:
                    emit_s(tgt)
                elif Jb + 1 < NB and tgt - nchunks < 2:
                    # pre-emit the next block's first S slots so the exp
                    # stream never gaps at the block boundary
                    nxt = Jb + 1
                    if tgt == nchunks:
                        pump(drain=(nxt, 0))
                    store = carry.setdefault(nxt, {})
                    make_emit_s(nxt, qr_tiles[nxt], store)(tgt - nchunks)
                emit_av(g)
                if 2 <= g < nchunks - 1:
                    pump()

            # normalize: oT[h] = o * (1/l). 1/l comes straight from the PSUM
            # ones-row via reciprocal_approx_fast; gpsimd broadcasts it.
            oT = opool.tile([128, QB], F16, tag="oT", name=f"oT_{Jb}")
            o_tiles[Jb] = oT
            last = Jb == NB - 1
            for h in range(2):
                r = 64 * h
                l_sb = spool.tile([1, QB], F32, tag="lsb")
                # the l copy rides the ACT queue: it lands exactly in ACT's
                # block-boundary gap while the DVE is busy with casts
                nc.scalar.copy(l_sb[:], o_ps[h][64:65, :])
                rb = spool.tile([1, QB], F32, tag="rb")
                nc.vector.reciprocal_approx_fast(rb[:], l_sb[:])
                bc = spool.tile([64, QB], F32, tag="bc")
                nc.gpsimd.partition_broadcast(bc[:], rb[:])
                nc.vector.tensor_mul(oT[r : r + 64, :], o_ps[h][0:64, :], bc[:])

            if last:
                for p in y_pieces(Jb, tail=True):
                    p()
            else:
                fillers.extend(((-1, -1), p) for p in y_pieces(Jb))

        while fillers:
            fillers.popleft()[1]()

    nc.compile()
    return nc


# ---------------- host-side wrapper ----------------

_CACHE = {}


def _get_nc(T):
    if T not in _CACHE:
        _CACHE[T] = build_kernel(T)
    return _CACHE[T]


def _host_prep(x, cos, sin, Wq, Wk, Wv, Wp):
    T = x.shape[1]
    cosT = np.ascontiguousarray(cos.T).astype(np.float32)  # [32, T]
    sinT = np.ascontiguousarray(sin.T).astype(np.float32)
    ccT = np.concatenate([cosT] * 4, axis=0).astype(np.float16)  # [128, T]
    sgn = np.where((np.arange(128) % 64) < 32, 1.0, -1.0)[:, None].astype(np.float32)
    ssT = (np.concatenate([sinT] * 4, axis=0) * sgn).astype(np.float16)
    rr = np.arange(128)[:, None]
    qq = np.arange(128)[None, :]
    # -500 on strictly-upper (masked) slots; exp((s-500)/8) underflows to 0
    trib = np.where(qq < rr, -500.0, 0.0).astype(np.float16)
    masks = np.stack([trib, trib], axis=1)  # [128, 2, 128] (one per head)
    idm = np.eye(128, dtype=np.float16)
    # qb = perm.T @ qa: qb[d] = qa[swap(d)], swap = +-32 within each 64-row head
    dd = np.arange(128)
    swap = np.where((dd % 64) < 32, dd + 32, dd - 32)
    permM = np.zeros((128, 128), np.float16)
    permM[swap, dd] = 1.0

    in_maps = []
    for core in range(N_CORES):
        b, p = core // 4, core % 4
        hs = slice(128 * p, 128 * (p + 1))
        in_maps.append(
            {
                "xT": np.ascontiguousarray(x[b].T.astype(np.float16)),
                "ccT": ccT,
                "ssT": ssT,
                "wqT": np.ascontiguousarray(Wq[hs].T).astype(np.float16),
                "wkT": np.ascontiguousarray(Wk[hs].T).astype(np.float16),
                "wvT": np.ascontiguousarray(Wv[hs].T).astype(np.float16),
                "wpT": np.ascontiguousarray(Wp[:, hs].T.astype(np.float16)),
                "masks": masks,
                "idm": idm,
                "perm": permM,
            }
        )
    return in_maps


def kernel(x, cos, sin, Wq, Wk, Wv, Wp, _trace=False, _nc=None):
    x = np.asarray(x)
    T = x.shape[1]
    nc = _nc if _nc is not None else _get_nc(T)
    in_maps = _host_prep(
        x, np.asarray(cos), np.asarray(sin),
        np.asarray(Wq), np.asarray(Wk), np.asarray(Wv), np.asarray(Wp),
    )
    res = run_bass_kernel_spmd(nc, in_maps, list(range(N_CORES)), trace=_trace)
    y = np.zeros((B, T, C), np.float32)
    for core in range(N_CORES):
        y[core // 4] += res.results[core]["yT"].T.astype(np.float32)
    kernel.last_results = res
    return y
